# revision 2
# baseline (speedup 1.0000x reference)
"""3-layer GCN + img@pair_embed.T for Trainium2, distributed over 8 NeuronCores.

Strategy (destination-sharded graph parallelism):
  - Each core owns a contiguous slab of destination nodes (3567, padded to 3584).
  - Edges (plus self-loops) are bucketed per 256-destination tile, sorted by
    destination, padded to chunks of 128. Host builds, per edge chunk, a dense
    [128 edges x 256 dests] selection matrix S holding the GCN norm
    coefficients, so segment-sum aggregation becomes TensorE matmuls:
        aggT[f, d] += G[e, f].T @ S[e, d]      (G = gathered source rows)
  - GCN identity A@(X W) == (A@X) W lets layer 1 aggregate 512-wide inputs
    (not 2048-wide outputs).
  - The final  img @ pair_embed.T  folds into layer 3: with
    W3img = W3 @ img.T  [2048, 64], Q = h2 @ W3img, the layer-3 aggregation
    A @ Q directly produces output columns. Layer 3 aggregates 64-wide.
  - Two AllGathers move activations between layers: h1 (4x [3584,512] f32) and
    Q ([3584,64] f32).
  - Matmuls run as float32r (full PE rate at free-dim>=256, ~12-bit mantissa);
    all operands are fp32r-rounded (host-side for inputs, on-engine for
    activations).
"""

import numpy as np

from concourse import bacc, bass, mybir
from concourse import tile as tile_mod
from concourse.bass_utils import run_bass_kernel_spmd

# Problem shapes (hardcoded per spec nn_GraphModel_26268019982828)
N = 28535
E = 113000
D = 512
H = 2048
B = 64
N_SKIP = 115 + 245  # attrs + objs; pair nodes are N_SKIP..N-1

NCORES = 8
NODES_PER = -(-N // NCORES)  # 3567
P = 128
DT = 256  # destination tile width
NDT = 14  # dest tiles per core
SLAB = NDT * DT  # 3584 padded dests per core
NFI1 = D // P  # 4 input feature chunks (layer 1)
NFI2 = H // P  # 16 feature chunks (2048)
NG = 4  # h1 column groups (4 x 512)
NFO2 = H // P  # 16 output chunks for layer2 GEMM

f32 = mybir.dt.float32
f32r = mybir.dt.float32r
i32 = mybir.dt.int32


def _round_fp32r(x: np.ndarray) -> np.ndarray:
    """Round-to-nearest-even fp32 -> fp32r (11-bit mantissa), numpy."""
    u = np.ascontiguousarray(x, dtype=np.float32).view(np.uint32)
    r = u + (0x7FF + ((u >> 12) & np.uint32(1)))
    r &= np.uint32(0xFFFFF000)
    return r.view(np.float32)


def _preprocess(edge_index: np.ndarray):
    """Sort/bucket edges by destination; build gather indices + S blocks.

    Returns (ECH, idxA, idxB, S) with
      idxA [NCORES, NDT, P, ECH] int32 — source node ids (original numbering)
      idxB [NCORES, NDT, P, ECH] int32 — source ids remapped to slab rows
      S    [NCORES, NDT, P, ECH*DT] float32 (fp32r-rounded) — norm matrix
    """
    src = np.concatenate([edge_index[0], np.arange(N, dtype=np.int64)])
    dst = np.concatenate([edge_index[1], np.arange(N, dtype=np.int64)])
    deg = np.bincount(dst, minlength=N).astype(np.float32)  # includes loops
    dinv = (1.0 / np.sqrt(deg)).astype(np.float32)
    norm = (dinv[src] * dinv[dst]).astype(np.float32)

    core = (dst // NODES_PER).astype(np.int64)
    local = (dst - core * NODES_PER).astype(np.int64)
    t_idx = local // DT
    d_local = local % DT
    bucket = core * NDT + t_idx  # global bucket id per edge

    order = np.argsort(bucket, kind="stable")
    src_s = src[order]
    bucket_s = bucket[order]
    dl_s = d_local[order]
    norm_s = norm[order]

    counts = np.bincount(bucket_s, minlength=NCORES * NDT)
    ECH = int(-(-counts.max() // P))

    idxA = np.zeros((NCORES, NDT, P, ECH), dtype=np.int32)
    idxB = np.zeros((NCORES, NDT, P, ECH), dtype=np.int32)
    S = np.zeros((NCORES, NDT, P, ECH * DT), dtype=np.float32)

    # position of each edge within its bucket
    starts = np.zeros(NCORES * NDT + 1, dtype=np.int64)
    np.cumsum(counts, out=starts[1:])
    pos = np.arange(len(bucket_s)) - starts[bucket_s]
    c_idx = pos // P  # edge chunk within bucket
    e_idx = pos % P  # partition row within chunk

    ci = bucket_s // NDT  # core
    ti = bucket_s % NDT  # dtile
    srcB = (src_s // NODES_PER) * SLAB + (src_s % NODES_PER)
    idxA[ci, ti, e_idx, c_idx] = src_s.astype(np.int32)
    idxB[ci, ti, e_idx, c_idx] = srcB.astype(np.int32)
    S[ci, ti, e_idx, c_idx * DT + dl_s] = norm_s
    S = _round_fp32r(S)
    return ECH, idxA, idxB, S


def _build(ECH: int, use_b1: bool, use_b2: bool):
    nc = bacc.Bacc("TRN2", target_bir_lowering=False, num_devices=NCORES)

    nodes_t = nc.dram_tensor("nodes", [N, D], f32r, kind="ExternalInput")
    w1_t = nc.dram_tensor("W1", [D, H], f32r, kind="ExternalInput")
    w2_t = nc.dram_tensor("W2", [H, H], f32r, kind="ExternalInput")
    w3i_t = nc.dram_tensor("W3img", [H, B], f32r, kind="ExternalInput")
    b1_t = nc.dram_tensor("b1", [1, H], f32r, kind="ExternalInput")
    b2_t = nc.dram_tensor("b2", [P, NFI2], f32r, kind="ExternalInput")
    idxA_t = nc.dram_tensor("idxA", [NDT, P, ECH], i32, kind="ExternalInput")
    idxB_t = nc.dram_tensor("idxB", [NDT, P, ECH], i32, kind="ExternalInput")
    s_tab = nc.dram_tensor("S", [NDT, P, ECH * DT], f32r, kind="ExternalInput")
    out_t = nc.dram_tensor("out", [B, SLAB], f32, kind="ExternalOutput")

    h1p = [nc.dram_tensor(f"h1p{g}", [SLAB, D], f32r) for g in range(NG)]
    h1pf = [
        nc.dram_tensor(f"h1pf{g}", [SLAB * NCORES, D], f32r, addr_space="Shared")
        for g in range(NG)
    ]
    q_slab = nc.dram_tensor("q_slab", [SLAB, B], f32r)
    q_full = nc.dram_tensor("q_full", [SLAB * NCORES, B], f32r, addr_space="Shared")

    rg = [list(range(NCORES))]

    with tile_mod.TileContext(nc) as tc:
        with (
            tc.tile_pool(name="w", bufs=16) as wp,
            tc.tile_pool(name="gio", bufs=4) as gp,
            tc.tile_pool(name="stab", bufs=2) as sp,
            tc.tile_pool(name="agg", bufs=16) as ap,
            tc.tile_pool(name="small", bufs=3) as mp,
            tc.tile_pool(name="consts", bufs=1) as cp,
            tc.tile_pool(name="pagg", bufs=4, space="PSUM") as pag,
            tc.tile_pool(name="pz", bufs=2, space="PSUM") as pzp,
            tc.tile_pool(name="pq", bufs=2, space="PSUM") as pqp,
        ):
            # --- resident weights ---
            w1sb = []
            for fi in range(NFI1):
                w = wp.tile([P, H], f32r, tag="w", name="wsb")
                nc.sync.dma_start(out=w[:], in_=w1_t[fi * P : (fi + 1) * P, :])
                w1sb.append(w)
            w2sb = []
            for fi in range(NFI2):
                w = wp.tile([P, H], f32r, tag="w", name="wsb")
                nc.sync.dma_start(out=w[:], in_=w2_t[fi * P : (fi + 1) * P, :])
                w2sb.append(w)
            w3sb = []
            for fo in range(NFI2):
                w = wp.tile([P, B], f32r, tag="w3", name="w3sb")
                nc.sync.dma_start(out=w[:], in_=w3i_t[fo * P : (fo + 1) * P, :])
                w3sb.append(w)
            if use_b1:
                b1sb = cp.tile([1, H], f32r)
                nc.sync.dma_start(out=b1sb[:], in_=b1_t[:])
                ones1 = cp.tile([1, P], f32r)
                nc.gpsimd.memset(ones1[:], 1.0)
            if use_b2:
                b2sb = cp.tile([P, NFI2], f32r)
                nc.sync.dma_start(out=b2sb[:], in_=b2_t[:])

            relu = mybir.ActivationFunctionType.Relu

            # ---------------- Layer 1 ----------------
            for t in range(NDT):
                idx_t = mp.tile([P, ECH], i32, tag="idx")
                nc.sync.dma_start(out=idx_t[:], in_=idxA_t[t])
                s_t = sp.tile([P, ECH * DT], f32r, tag="s")
                nc.sync.dma_start(out=s_t[:], in_=s_tab[t])

                pa = [pag.tile([P, DT], f32, tag="pagg", name="pa") for _ in range(NFI1)]
                for c in range(ECH):
                    g = gp.tile([P, D], f32r, tag="g")
                    nc.gpsimd.indirect_dma_start(
                        out=g[:],
                        out_offset=None,
                        in_=nodes_t[:],
                        in_offset=bass.IndirectOffsetOnAxis(
                            ap=idx_t[:, c : c + 1], axis=0
                        ),
                    )
                    for fi in range(NFI1):
                        nc.tensor.matmul(
                            out=pa[fi][:],
                            lhsT=g[:, fi * P : (fi + 1) * P],
                            rhs=s_t[:, c * DT : (c + 1) * DT],
                            start=(c == 0),
                            stop=(c == ECH - 1),
                        )
                aggT = []
                for fi in range(NFI1):
                    a = ap.tile([P, DT], f32r, tag="aggT", name="aggTt")
                    nc.vector.tensor_copy(out=a[:], in_=pa[fi][:])
                    aggT.append(a)

                for dh in range(2):
                    for fo in range(NG):  # 4 output blocks of 512
                        pz = pzp.tile([P, D], f32, tag="pz")
                        if use_b1:
                            nc.tensor.matmul(
                                out=pz[:],
                                lhsT=ones1[:1, :],
                                rhs=b1sb[:1, fo * D : (fo + 1) * D],
                                start=True,
                                stop=False,
                            )
                        for fi in range(NFI1):
                            nc.tensor.matmul(
                                out=pz[:],
                                lhsT=aggT[fi][:, dh * P : (dh + 1) * P],
                                rhs=w1sb[fi][:, fo * D : (fo + 1) * D],
                                start=(fi == 0 and not use_b1),
                                stop=(fi == NFI1 - 1),
                            )
                        h_t = mp.tile([P, D], f32r, tag="hout")
                        nc.scalar.activation(out=h_t[:], in_=pz[:], func=relu)
                        nc.sync.dma_start(
                            out=h1p[fo][t * DT + dh * P : t * DT + (dh + 1) * P, :],
                            in_=h_t[:],
                        )

            for g_i in range(NG):
                nc.gpsimd.collective_compute(
                    "AllGather",
                    mybir.AluOpType.bypass,
                    replica_groups=rg,
                    ins=[h1p[g_i][:]],
                    outs=[h1pf[g_i][:]],
                )

            # ---------------- Layer 2 + Q ----------------
            for t in range(NDT):
                idx_t = mp.tile([P, ECH], i32, tag="idx")
                nc.sync.dma_start(out=idx_t[:], in_=idxB_t[t])
                s_t = sp.tile([P, ECH * DT], f32r, tag="s")
                nc.sync.dma_start(out=s_t[:], in_=s_tab[t])

                aggT = []
                for g_i in range(NG):
                    pa = [pag.tile([P, DT], f32, tag="pagg", name="pa") for _ in range(NFI1)]
                    for c in range(ECH):
                        g = gp.tile([P, D], f32r, tag="g")
                        nc.gpsimd.indirect_dma_start(
                            out=g[:],
                            out_offset=None,
                            in_=h1pf[g_i][:],
                            in_offset=bass.IndirectOffsetOnAxis(
                                ap=idx_t[:, c : c + 1], axis=0
                            ),
                        )
                        for j in range(NFI1):
                            nc.tensor.matmul(
                                out=pa[j][:],
                                lhsT=g[:, j * P : (j + 1) * P],
                                rhs=s_t[:, c * DT : (c + 1) * DT],
                                start=(c == 0),
                                stop=(c == ECH - 1),
                            )
                    for j in range(NFI1):
                        a = ap.tile([P, DT], f32r, tag="aggT", name="aggTt")
                        nc.vector.tensor_copy(out=a[:], in_=pa[j][:])
                        aggT.append(a)

                pq = [pqp.tile([P, B], f32, tag="pq", name="pq") for _ in range(2)]
                for fo in range(NFO2):
                    pz = pzp.tile([P, DT], f32, tag="pz")
                    for fi in range(NFI2):
                        nc.tensor.matmul(
                            out=pz[:],
                            lhsT=w2sb[fi][:, fo * P : (fo + 1) * P],
                            rhs=aggT[fi][:],
                            start=(fi == 0),
                            stop=(fi == NFI2 - 1),
                        )
                    h2t = mp.tile([P, DT], f32r, tag="h2")
                    if use_b2:
                        nc.scalar.activation(
                            out=h2t[:], in_=pz[:], func=relu,
                            bias=b2sb[:, fo : fo + 1],
                        )
                    else:
                        nc.scalar.activation(out=h2t[:], in_=pz[:], func=relu)
                    for dh in range(2):
                        nc.tensor.matmul(
                            out=pq[dh][:],
                            lhsT=h2t[:, dh * P : (dh + 1) * P],
                            rhs=w3sb[fo][:],
                            start=(fo == 0),
                            stop=(fo == NFO2 - 1),
                        )
                for dh in range(2):
                    qn = mp.tile([P, B], f32r, tag="qn")
                    nc.vector.tensor_copy(out=qn[:], in_=pq[dh][:])
                    nc.sync.dma_start(
                        out=q_slab[t * DT + dh * P : t * DT + (dh + 1) * P, :],
                        in_=qn[:],
                    )

            nc.gpsimd.collective_compute(
                "AllGather",
                mybir.AluOpType.bypass,
                replica_groups=rg,
                ins=[q_slab[:]],
                outs=[q_full[:]],
            )

            # ---------------- Layer 3 (= output) ----------------
            for t in range(NDT):
                idx_t = mp.tile([P, ECH], i32, tag="idx")
                nc.sync.dma_start(out=idx_t[:], in_=idxB_t[t])
                s_t = sp.tile([P, ECH * DT], f32r, tag="s")
                nc.sync.dma_start(out=s_t[:], in_=s_tab[t])

                pa = pag.tile([B, DT], f32, tag="pagg", name="pa3")
                for c in range(ECH):
                    g = gp.tile([P, B], f32r, tag="g")
                    nc.gpsimd.indirect_dma_start(
                        out=g[:],
                        out_offset=None,
                        in_=q_full[:],
                        in_offset=bass.IndirectOffsetOnAxis(
                            ap=idx_t[:, c : c + 1], axis=0
                        ),
                    )
                    nc.tensor.matmul(
                        out=pa[:],
                        lhsT=g[:],
                        rhs=s_t[:, c * DT : (c + 1) * DT],
                        start=(c == 0),
                        stop=(c == ECH - 1),
                    )
                o_t = mp.tile([B, DT], f32, tag="ot")
                nc.vector.tensor_copy(out=o_t[:], in_=pa[:])
                nc.sync.dma_start(out=out_t[:, t * DT : (t + 1) * DT], in_=o_t[:])

    nc.finalize()
    return nc


_CACHE: dict = {}


def kernel(**inputs: np.ndarray) -> np.ndarray:
    nodes = np.asarray(inputs["nodes"], dtype=np.float32)
    edge_index = np.asarray(inputs["edge_index"])
    img = np.asarray(inputs["img"], dtype=np.float32)
    W1 = np.asarray(inputs["W1"], dtype=np.float32)
    b1 = np.asarray(inputs["b1"], dtype=np.float32)
    W2 = np.asarray(inputs["W2"], dtype=np.float32)
    b2 = np.asarray(inputs["b2"], dtype=np.float32)
    W3 = np.asarray(inputs["W3"], dtype=np.float32)
    b3 = np.asarray(inputs["b3"], dtype=np.float32)

    ECH, idxA, idxB, S = _preprocess(edge_index)
    use_b1 = bool(np.any(b1))
    use_b2 = bool(np.any(b2))

    key = (ECH, use_b1, use_b2)
    if key not in _CACHE:
        _CACHE[key] = _build(ECH, use_b1, use_b2)
    nc = _CACHE[key]

    w3img = _round_fp32r(W3.astype(np.float32) @ img.astype(np.float32).T)  # [H, B]
    outbias = img @ b3  # [B]

    nodes_r = _round_fp32r(nodes)
    w1_r = _round_fp32r(W1)
    w2_r = _round_fp32r(W2)
    b1_r = _round_fp32r(b1.reshape(1, H))
    b2_r = _round_fp32r(np.ascontiguousarray(b2.reshape(NFI2, P).T))

    in_maps = []
    for k in range(NCORES):
        in_maps.append(
            {
                "nodes": nodes_r,
                "W1": w1_r,
                "W2": w2_r,
                "W3img": w3img,
                "b1": b1_r,
                "b2": b2_r,
                "idxA": np.ascontiguousarray(idxA[k]),
                "idxB": np.ascontiguousarray(idxB[k]),
                "S": np.ascontiguousarray(S[k]),
            }
        )

    res = run_bass_kernel_spmd(nc, in_maps, core_ids=list(range(NCORES)))

    full = np.concatenate([res.results[k]["out"] for k in range(NCORES)], axis=1)
    n_ids = np.arange(N_SKIP, N)
    cols = (n_ids // NODES_PER) * SLAB + (n_ids % NODES_PER)
    out = full[:, cols] + outbias[:, None]
    return out.astype(np.float32)


if __name__ == "__main__":
    # quick self-exercise with random data
    rng = np.random.default_rng(0)
    ins = {
        "nodes": rng.standard_normal((N, D)).astype(np.float32),
        "edge_index": rng.integers(0, N, size=(2, E)).astype(np.int64),
        "img": rng.standard_normal((B, D)).astype(np.float32),
        "W1": (rng.standard_normal((D, H)) * 0.02).astype(np.float32),
        "b1": np.zeros(H, np.float32),
        "W2": (rng.standard_normal((H, H)) * 0.02).astype(np.float32),
        "b2": np.zeros(H, np.float32),
        "W3": (rng.standard_normal((H, D)) * 0.02).astype(np.float32),
        "b3": np.zeros(D, np.float32),
    }
    out = kernel(**ins)
    print("out", out.shape, out.dtype, np.abs(out).mean())


# revision 6
# speedup vs baseline: 1.0915x; 1.0915x over previous
"""3-layer GCN + img@pair_embed.T for Trainium2, distributed over 8 NeuronCores.

Strategy (destination-sharded graph parallelism):
  - Each core owns a contiguous slab of destination nodes (3567, padded to 3584).
  - Edges (plus self-loops) are bucketed per 256-destination tile, sorted by
    destination, padded to chunks of 128. Host builds, per edge chunk, a dense
    [128 edges x 256 dests] selection matrix S holding the GCN norm
    coefficients, so segment-sum aggregation becomes TensorE matmuls:
        aggT[f, d] += G[e, f].T @ S[e, d]      (G = gathered source rows)
  - GCN identity A@(X W) == (A@X) W lets layer 1 aggregate 512-wide inputs
    (not 2048-wide outputs).
  - The final  img @ pair_embed.T  folds into layer 3: with
    W3img = W3 @ img.T  [2048, 64], Q = h2 @ W3img, the layer-3 aggregation
    A @ Q directly produces output columns. Layer 3 aggregates 64-wide.
  - Two AllGathers move activations between layers: h1 (4x [3584,512] f32) and
    Q ([3584,64] f32).
  - Matmuls run as float32r (full PE rate at free-dim>=256, ~12-bit mantissa);
    all operands are fp32r-rounded (host-side for inputs, on-engine for
    activations).
"""

import numpy as np

from concourse import bacc, bass, mybir
from concourse import tile as tile_mod
from concourse.bass_utils import run_bass_kernel_spmd

# Problem shapes (hardcoded per spec nn_GraphModel_26268019982828)
N = 28535
E = 113000
D = 512
H = 2048
B = 64
N_SKIP = 115 + 245  # attrs + objs; pair nodes are N_SKIP..N-1

NCORES = 8
NODES_PER = -(-N // NCORES)  # 3567
P = 128
DT = 256  # destination tile width
NDT = 14  # dest tiles per core
SLAB = NDT * DT  # 3584 padded dests per core
NFI1 = D // P  # 4 input feature chunks (layer 1)
NFI2 = H // P  # 16 feature chunks (2048)
NG = 4  # h1 column groups (4 x 512)
NFO2 = H // P  # 16 output chunks for layer2 GEMM

f32 = mybir.dt.float32
f32r = mybir.dt.float32r
bf16 = mybir.dt.bfloat16
i32 = mybir.dt.int32
FEAT_BF16 = True  # False -> fp32r feature path (safer numerics, ~2x memory)
FEAT = bf16 if FEAT_BF16 else f32r  # gathered/exchanged activations + S blocks


def _round_fp32r(x: np.ndarray) -> np.ndarray:
    """Round-to-nearest-even fp32 -> fp32r (11-bit mantissa), numpy."""
    u = np.ascontiguousarray(x, dtype=np.float32).view(np.uint32)
    r = u + (0x7FF + ((u >> 12) & np.uint32(1)))
    r &= np.uint32(0xFFFFF000)
    return r.view(np.float32)


def _preprocess(edge_index: np.ndarray):
    """Sort/bucket edges by destination; build gather indices + S blocks.

    Returns (ECH, idxA, idxB, S) with
      idxA [NCORES, NDT, P, ECH] int32 — source node ids (original numbering)
      idxB [NCORES, NDT, P, ECH] int32 — source ids remapped to slab rows
      S    [NCORES, NDT, P, ECH*DT] float32 (fp32r-rounded) — norm matrix
    """
    src = np.concatenate([edge_index[0], np.arange(N, dtype=np.int64)])
    dst = np.concatenate([edge_index[1], np.arange(N, dtype=np.int64)])
    deg = np.bincount(dst, minlength=N).astype(np.float32)  # includes loops
    dinv = (1.0 / np.sqrt(deg)).astype(np.float32)
    norm = (dinv[src] * dinv[dst]).astype(np.float32)

    core = (dst // NODES_PER).astype(np.int64)
    local = (dst - core * NODES_PER).astype(np.int64)
    t_idx = local // DT
    d_local = local % DT
    bucket = core * NDT + t_idx  # global bucket id per edge

    order = np.argsort(bucket, kind="stable")
    src_s = src[order]
    bucket_s = bucket[order]
    dl_s = d_local[order]
    norm_s = norm[order]

    counts = np.bincount(bucket_s, minlength=NCORES * NDT)
    ECH = int(-(-counts.max() // P))

    idxA = np.zeros((NCORES, NDT, P, ECH), dtype=np.int32)
    idxB = np.zeros((NCORES, NDT, P, ECH), dtype=np.int32)
    S = np.zeros((NCORES, NDT, P, ECH * DT), dtype=np.float32)

    # position of each edge within its bucket
    starts = np.zeros(NCORES * NDT + 1, dtype=np.int64)
    np.cumsum(counts, out=starts[1:])
    pos = np.arange(len(bucket_s)) - starts[bucket_s]
    c_idx = pos // P  # edge chunk within bucket
    e_idx = pos % P  # partition row within chunk

    ci = bucket_s // NDT  # core
    ti = bucket_s % NDT  # dtile
    srcB = (src_s // NODES_PER) * SLAB + (src_s % NODES_PER)
    idxA[ci, ti, e_idx, c_idx] = src_s.astype(np.int32)
    idxB[ci, ti, e_idx, c_idx] = srcB.astype(np.int32)
    S[ci, ti, e_idx, c_idx * DT + dl_s] = norm_s
    S = _round_fp32r(S)
    return ECH, idxA, idxB, S


def _build(ECH: int, use_b1: bool, use_b2: bool, _phases: int = 3):
    nc = bacc.Bacc("TRN2", target_bir_lowering=False, num_devices=NCORES)

    nodes_t = nc.dram_tensor("nodes", [N, D], FEAT, kind="ExternalInput")
    w1_t = nc.dram_tensor("W1", [D, H], f32r, kind="ExternalInput")
    w2_t = nc.dram_tensor("W2", [H, H], f32r, kind="ExternalInput")
    w3i_t = nc.dram_tensor("W3img", [H, B], f32r, kind="ExternalInput")
    b1_t = nc.dram_tensor("b1", [1, H], f32r, kind="ExternalInput")
    b2_t = nc.dram_tensor("b2", [P, NFI2], f32r, kind="ExternalInput")
    idxA_t = nc.dram_tensor("idxA", [NDT, P, ECH], i32, kind="ExternalInput")
    idxB_t = nc.dram_tensor("idxB", [NDT, P, ECH], i32, kind="ExternalInput")
    s_tab = nc.dram_tensor("S", [NDT, P, ECH * DT], FEAT, kind="ExternalInput")
    out_t = nc.dram_tensor("out", [B, SLAB], f32, kind="ExternalOutput")

    h1p = [nc.dram_tensor(f"h1p{g}", [SLAB, D], FEAT) for g in range(NG)]
    h1pf = [
        nc.dram_tensor(f"h1pf{g}", [SLAB * NCORES, D], FEAT, addr_space="Shared")
        for g in range(NG)
    ]
    q_slab = nc.dram_tensor("q_slab", [SLAB, B], FEAT)
    q_full = nc.dram_tensor("q_full", [SLAB * NCORES, B], FEAT, addr_space="Shared")

    rg = [list(range(NCORES))]

    with tile_mod.TileContext(nc) as tc:
        with (
            tc.tile_pool(name="w", bufs=16) as wp,
            tc.tile_pool(name="gio", bufs=4) as gp,
            tc.tile_pool(name="stab", bufs=2) as sp,
            tc.tile_pool(name="agg", bufs=16) as ap,
            tc.tile_pool(name="small", bufs=3) as mp,
            tc.tile_pool(name="consts", bufs=1) as cp,
            tc.tile_pool(name="pagg", bufs=4, space="PSUM") as pag,
            tc.tile_pool(name="pz", bufs=2, space="PSUM") as pzp,
            tc.tile_pool(name="pq", bufs=2, space="PSUM") as pqp,
        ):
            # --- resident weights ---
            w1sb = []
            for fi in range(NFI1):
                w = wp.tile([P, H], f32r, tag="w", name="wsb")
                nc.sync.dma_start(out=w[:], in_=w1_t[fi * P : (fi + 1) * P, :])
                w1sb.append(w)
            w2sb = []
            for fi in range(NFI2):
                w = wp.tile([P, H], f32r, tag="w", name="wsb")
                nc.sync.dma_start(out=w[:], in_=w2_t[fi * P : (fi + 1) * P, :])
                w2sb.append(w)
            w3sb = []
            for fo in range(NFI2):
                w = wp.tile([P, B], f32r, tag="w3", name="w3sb")
                nc.sync.dma_start(out=w[:], in_=w3i_t[fo * P : (fo + 1) * P, :])
                w3sb.append(w)
            if use_b1:
                b1sb = cp.tile([1, H], f32r)
                nc.sync.dma_start(out=b1sb[:], in_=b1_t[:])
                ones1 = cp.tile([1, P], f32r)
                nc.gpsimd.memset(ones1[:], 1.0)
            if use_b2:
                b2sb = cp.tile([P, NFI2], f32r)
                nc.sync.dma_start(out=b2sb[:], in_=b2_t[:])

            relu = mybir.ActivationFunctionType.Relu

            # ---------------- Layer 1 ----------------
            for t in range(NDT if _phases >= 1 else 0):
                idx_t = mp.tile([P, ECH], i32, tag="idx")
                nc.sync.dma_start(out=idx_t[:], in_=idxA_t[t])
                s_t = sp.tile([P, ECH * DT], FEAT, tag="s")
                nc.sync.dma_start(out=s_t[:], in_=s_tab[t])

                pa = [pag.tile([P, DT], f32, tag="pagg", name="pa") for _ in range(NFI1)]
                for c in range(ECH):
                    g = gp.tile([P, D], FEAT, tag="g")
                    nc.gpsimd.indirect_dma_start(
                        out=g[:],
                        out_offset=None,
                        in_=nodes_t[:],
                        in_offset=bass.IndirectOffsetOnAxis(
                            ap=idx_t[:, c : c + 1], axis=0
                        ),
                    )
                    for fi in range(NFI1):
                        nc.tensor.matmul(
                            out=pa[fi][:],
                            lhsT=g[:, fi * P : (fi + 1) * P],
                            rhs=s_t[:, c * DT : (c + 1) * DT],
                            start=(c == 0),
                            stop=(c == ECH - 1),
                        )
                aggT = []
                for fi in range(NFI1):
                    a = ap.tile([P, DT], f32r, tag="aggT", name="aggTt")
                    nc.vector.tensor_copy(out=a[:], in_=pa[fi][:])
                    aggT.append(a)

                for dh in range(2):
                    for fo in range(NG):  # 4 output blocks of 512
                        pz = pzp.tile([P, D], f32, tag="pz")
                        if use_b1:
                            nc.tensor.matmul(
                                out=pz[:],
                                lhsT=ones1[:1, :],
                                rhs=b1sb[:1, fo * D : (fo + 1) * D],
                                start=True,
                                stop=False,
                            )
                        for fi in range(NFI1):
                            nc.tensor.matmul(
                                out=pz[:],
                                lhsT=aggT[fi][:, dh * P : (dh + 1) * P],
                                rhs=w1sb[fi][:, fo * D : (fo + 1) * D],
                                start=(fi == 0 and not use_b1),
                                stop=(fi == NFI1 - 1),
                            )
                        h_t = mp.tile([P, D], FEAT, tag="hout")
                        nc.scalar.activation(out=h_t[:], in_=pz[:], func=relu)
                        nc.sync.dma_start(
                            out=h1p[fo][t * DT + dh * P : t * DT + (dh + 1) * P, :],
                            in_=h_t[:],
                        )

            for g_i in range(NG if _phases >= 1.5 else 0):
                nc.gpsimd.collective_compute(
                    "AllGather",
                    mybir.AluOpType.bypass,
                    replica_groups=rg,
                    ins=[h1p[g_i][:]],
                    outs=[h1pf[g_i][:]],
                )

            # ---------------- Layer 2 + Q ----------------
            for t in range(NDT if _phases >= 2 else 0):
                idx_t = mp.tile([P, ECH], i32, tag="idx")
                nc.sync.dma_start(out=idx_t[:], in_=idxB_t[t])
                s_t = sp.tile([P, ECH * DT], FEAT, tag="s")
                nc.sync.dma_start(out=s_t[:], in_=s_tab[t])

                aggT = []
                for g_i in range(NG):
                    pa = [pag.tile([P, DT], f32, tag="pagg", name="pa") for _ in range(NFI1)]
                    for c in range(ECH):
                        g = gp.tile([P, D], FEAT, tag="g")
                        nc.gpsimd.indirect_dma_start(
                            out=g[:],
                            out_offset=None,
                            in_=h1pf[g_i][:],
                            in_offset=bass.IndirectOffsetOnAxis(
                                ap=idx_t[:, c : c + 1], axis=0
                            ),
                        )
                        for j in range(NFI1):
                            nc.tensor.matmul(
                                out=pa[j][:],
                                lhsT=g[:, j * P : (j + 1) * P],
                                rhs=s_t[:, c * DT : (c + 1) * DT],
                                start=(c == 0),
                                stop=(c == ECH - 1),
                            )
                    for j in range(NFI1):
                        a = ap.tile([P, DT], f32r, tag="aggT", name="aggTt")
                        nc.vector.tensor_copy(out=a[:], in_=pa[j][:])
                        aggT.append(a)

                pq = [pqp.tile([P, B], f32, tag="pq", name="pq") for _ in range(2)]
                for fo in range(NFO2):
                    pz = pzp.tile([P, DT], f32, tag="pz")
                    for fi in range(NFI2):
                        nc.tensor.matmul(
                            out=pz[:],
                            lhsT=w2sb[fi][:, fo * P : (fo + 1) * P],
                            rhs=aggT[fi][:],
                            start=(fi == 0),
                            stop=(fi == NFI2 - 1),
                        )
                    h2t = mp.tile([P, DT], f32r, tag="h2")
                    if use_b2:
                        nc.scalar.activation(
                            out=h2t[:], in_=pz[:], func=relu,
                            bias=b2sb[:, fo : fo + 1],
                        )
                    else:
                        nc.scalar.activation(out=h2t[:], in_=pz[:], func=relu)
                    for dh in range(2):
                        nc.tensor.matmul(
                            out=pq[dh][:],
                            lhsT=h2t[:, dh * P : (dh + 1) * P],
                            rhs=w3sb[fo][:],
                            start=(fo == 0),
                            stop=(fo == NFO2 - 1),
                        )
                for dh in range(2):
                    qn = mp.tile([P, B], FEAT, tag="qn")
                    nc.vector.tensor_copy(out=qn[:], in_=pq[dh][:])
                    nc.sync.dma_start(
                        out=q_slab[t * DT + dh * P : t * DT + (dh + 1) * P, :],
                        in_=qn[:],
                    )

            if _phases >= 2:
                nc.gpsimd.collective_compute(
                    "AllGather",
                    mybir.AluOpType.bypass,
                    replica_groups=rg,
                    ins=[q_slab[:]],
                    outs=[q_full[:]],
                )

            # ---------------- Layer 3 (= output) ----------------
            for t in range(NDT if _phases >= 3 else 0):
                idx_t = mp.tile([P, ECH], i32, tag="idx")
                nc.sync.dma_start(out=idx_t[:], in_=idxB_t[t])
                s_t = sp.tile([P, ECH * DT], FEAT, tag="s")
                nc.sync.dma_start(out=s_t[:], in_=s_tab[t])

                pa = pag.tile([B, DT], f32, tag="pagg", name="pa3")
                for c in range(ECH):
                    g = gp.tile([P, B], FEAT, tag="g")
                    nc.gpsimd.indirect_dma_start(
                        out=g[:],
                        out_offset=None,
                        in_=q_full[:],
                        in_offset=bass.IndirectOffsetOnAxis(
                            ap=idx_t[:, c : c + 1], axis=0
                        ),
                    )
                    nc.tensor.matmul(
                        out=pa[:],
                        lhsT=g[:],
                        rhs=s_t[:, c * DT : (c + 1) * DT],
                        start=(c == 0),
                        stop=(c == ECH - 1),
                    )
                o_t = mp.tile([B, DT], f32, tag="ot")
                nc.vector.tensor_copy(out=o_t[:], in_=pa[:])
                nc.sync.dma_start(out=out_t[:, t * DT : (t + 1) * DT], in_=o_t[:])

    nc.finalize()
    return nc


_CACHE: dict = {}


def kernel(**inputs: np.ndarray) -> np.ndarray:
    nodes = np.asarray(inputs["nodes"], dtype=np.float32)
    edge_index = np.asarray(inputs["edge_index"])
    img = np.asarray(inputs["img"], dtype=np.float32)
    W1 = np.asarray(inputs["W1"], dtype=np.float32)
    b1 = np.asarray(inputs["b1"], dtype=np.float32)
    W2 = np.asarray(inputs["W2"], dtype=np.float32)
    b2 = np.asarray(inputs["b2"], dtype=np.float32)
    W3 = np.asarray(inputs["W3"], dtype=np.float32)
    b3 = np.asarray(inputs["b3"], dtype=np.float32)

    ECH, idxA, idxB, S = _preprocess(edge_index)
    if FEAT_BF16:
        S = S.astype(__import__("ml_dtypes").bfloat16)
    use_b1 = bool(np.any(b1))
    use_b2 = bool(np.any(b2))

    key = (ECH, use_b1, use_b2)
    if key not in _CACHE:
        _CACHE[key] = _build(ECH, use_b1, use_b2)
    nc = _CACHE[key]

    w3img = _round_fp32r(W3.astype(np.float32) @ img.astype(np.float32).T)  # [H, B]
    outbias = img @ b3  # [B]

    import ml_dtypes
    feat_np = ml_dtypes.bfloat16 if FEAT_BF16 else np.float32
    nodes_r = nodes.astype(feat_np) if FEAT_BF16 else _round_fp32r(nodes)
    w1_r = _round_fp32r(W1)
    w2_r = _round_fp32r(W2)
    b1_r = _round_fp32r(b1.reshape(1, H))
    b2_r = _round_fp32r(np.ascontiguousarray(b2.reshape(NFI2, P).T))

    in_maps = []
    for k in range(NCORES):
        in_maps.append(
            {
                "nodes": nodes_r,
                "W1": w1_r,
                "W2": w2_r,
                "W3img": w3img,
                "b1": b1_r,
                "b2": b2_r,
                "idxA": np.ascontiguousarray(idxA[k]),
                "idxB": np.ascontiguousarray(idxB[k]),
                "S": np.ascontiguousarray(S[k]),
            }
        )

    res = run_bass_kernel_spmd(nc, in_maps, core_ids=list(range(NCORES)))

    full = np.concatenate([res.results[k]["out"] for k in range(NCORES)], axis=1)
    n_ids = np.arange(N_SKIP, N)
    cols = (n_ids // NODES_PER) * SLAB + (n_ids % NODES_PER)
    out = full[:, cols] + outbias[:, None]
    return out.astype(np.float32)


if __name__ == "__main__":
    # quick self-exercise with random data
    rng = np.random.default_rng(0)
    ins = {
        "nodes": rng.standard_normal((N, D)).astype(np.float32),
        "edge_index": rng.integers(0, N, size=(2, E)).astype(np.int64),
        "img": rng.standard_normal((B, D)).astype(np.float32),
        "W1": (rng.standard_normal((D, H)) * 0.02).astype(np.float32),
        "b1": np.zeros(H, np.float32),
        "W2": (rng.standard_normal((H, H)) * 0.02).astype(np.float32),
        "b2": np.zeros(H, np.float32),
        "W3": (rng.standard_normal((H, D)) * 0.02).astype(np.float32),
        "b3": np.zeros(D, np.float32),
    }
    out = kernel(**ins)
    print("out", out.shape, out.dtype, np.abs(out).mean())


# revision 7
# speedup vs baseline: 7721.4823x; 7074.4881x over previous
"""3-layer GCN + img@pair_embed.T for Trainium2, distributed over 8 NeuronCores.

Strategy (destination-sharded graph parallelism):
  - Each core owns a contiguous slab of destination nodes (3567, padded to 3584).
  - Edges (plus self-loops) are bucketed per 256-destination tile, sorted by
    destination, padded to chunks of 128. Host builds, per edge chunk, a dense
    [128 edges x 256 dests] selection matrix S holding the GCN norm
    coefficients, so segment-sum aggregation becomes TensorE matmuls:
        aggT[f, d] += G[e, f].T @ S[e, d]      (G = gathered source rows)
  - GCN identity A@(X W) == (A@X) W lets layer 1 aggregate 512-wide inputs
    (not 2048-wide outputs).
  - The final  img @ pair_embed.T  folds into layer 3: with
    W3img = W3 @ img.T  [2048, 64], Q = h2 @ W3img, the layer-3 aggregation
    A @ Q directly produces output columns. Layer 3 aggregates 64-wide.
  - Five AllGathers move activations between layers: h1 (4x [3584,512]) and
    Q ([3584,64]).
  - Gathered/exchanged activations and S blocks travel as bf16 (FEAT_BF16
    toggle; halves HBM gather + collective bytes; measured rel err ~2.6e-3).
    GEMM weights are float32r (full PE rate at free-dim>=256, ~12-bit
    mantissa, host-pre-rounded); PSUM accumulation is always fp32.
"""

import numpy as np

from concourse import bacc, bass, mybir
from concourse import tile as tile_mod
from concourse.bass_utils import run_bass_kernel_spmd

# Problem shapes (hardcoded per spec nn_GraphModel_26268019982828)
N = 28535
E = 113000
D = 512
H = 2048
B = 64
N_SKIP = 115 + 245  # attrs + objs; pair nodes are N_SKIP..N-1

NCORES = 8
NODES_PER = -(-N // NCORES)  # 3567
P = 128
DT = 256  # destination tile width
NDT = 14  # dest tiles per core
SLAB = NDT * DT  # 3584 padded dests per core
NFI1 = D // P  # 4 input feature chunks (layer 1)
NFI2 = H // P  # 16 feature chunks (2048)
NG = 4  # h1 column groups (4 x 512)
NFO2 = H // P  # 16 output chunks for layer2 GEMM

f32 = mybir.dt.float32
f32r = mybir.dt.float32r
bf16 = mybir.dt.bfloat16
i32 = mybir.dt.int32
FEAT_BF16 = True  # False -> fp32r feature path (safer numerics, ~2x memory)
FEAT = bf16 if FEAT_BF16 else f32r  # gathered/exchanged activations + S blocks


def _round_fp32r(x: np.ndarray) -> np.ndarray:
    """Round-to-nearest-even fp32 -> fp32r (11-bit mantissa), numpy."""
    u = np.ascontiguousarray(x, dtype=np.float32).view(np.uint32)
    r = u + (0x7FF + ((u >> 12) & np.uint32(1)))
    r &= np.uint32(0xFFFFF000)
    return r.view(np.float32)


def _preprocess(edge_index: np.ndarray):
    """Sort/bucket edges by destination; build gather indices + S blocks.

    Returns (ECH, idxA, idxB, S) with
      idxA [NCORES, NDT, P, ECH] int32 — source node ids (original numbering)
      idxB [NCORES, NDT, P, ECH] int32 — source ids remapped to slab rows
      S    [NCORES, NDT, P, ECH*DT] float32 (fp32r-rounded) — norm matrix
    """
    src = np.concatenate([edge_index[0], np.arange(N, dtype=np.int64)])
    dst = np.concatenate([edge_index[1], np.arange(N, dtype=np.int64)])
    deg = np.bincount(dst, minlength=N).astype(np.float32)  # includes loops
    dinv = (1.0 / np.sqrt(deg)).astype(np.float32)
    norm = (dinv[src] * dinv[dst]).astype(np.float32)

    core = (dst // NODES_PER).astype(np.int64)
    local = (dst - core * NODES_PER).astype(np.int64)
    t_idx = local // DT
    d_local = local % DT
    bucket = core * NDT + t_idx  # global bucket id per edge

    order = np.argsort(bucket, kind="stable")
    src_s = src[order]
    bucket_s = bucket[order]
    dl_s = d_local[order]
    norm_s = norm[order]

    counts = np.bincount(bucket_s, minlength=NCORES * NDT)
    ECH = int(-(-counts.max() // P))

    idxA = np.zeros((NCORES, NDT, P, ECH), dtype=np.int32)
    idxB = np.zeros((NCORES, NDT, P, ECH), dtype=np.int32)
    S = np.zeros((NCORES, NDT, P, ECH * DT), dtype=np.float32)

    # position of each edge within its bucket
    starts = np.zeros(NCORES * NDT + 1, dtype=np.int64)
    np.cumsum(counts, out=starts[1:])
    pos = np.arange(len(bucket_s)) - starts[bucket_s]
    c_idx = pos // P  # edge chunk within bucket
    e_idx = pos % P  # partition row within chunk

    ci = bucket_s // NDT  # core
    ti = bucket_s % NDT  # dtile
    srcB = (src_s // NODES_PER) * SLAB + (src_s % NODES_PER)
    idxA[ci, ti, e_idx, c_idx] = src_s.astype(np.int32)
    idxB[ci, ti, e_idx, c_idx] = srcB.astype(np.int32)
    S[ci, ti, e_idx, c_idx * DT + dl_s] = norm_s
    S = _round_fp32r(S)
    return ECH, idxA, idxB, S


def _build(ECH: int, use_b1: bool, use_b2: bool, _phases: int = 3):
    nc = bacc.Bacc("TRN2", target_bir_lowering=False, num_devices=NCORES)

    nodes_t = nc.dram_tensor("nodes", [N, D], FEAT, kind="ExternalInput")
    w1_t = nc.dram_tensor("W1", [D, H], f32r, kind="ExternalInput")
    w2_t = nc.dram_tensor("W2", [H, H], f32r, kind="ExternalInput")
    w3i_t = nc.dram_tensor("W3img", [H, B], f32r, kind="ExternalInput")
    b1_t = nc.dram_tensor("b1", [1, H], f32r, kind="ExternalInput")
    b2_t = nc.dram_tensor("b2", [P, NFI2], f32r, kind="ExternalInput")
    idxA_t = nc.dram_tensor("idxA", [NDT, P, ECH], i32, kind="ExternalInput")
    idxB_t = nc.dram_tensor("idxB", [NDT, P, ECH], i32, kind="ExternalInput")
    s_tab = nc.dram_tensor("S", [NDT, P, ECH * DT], FEAT, kind="ExternalInput")
    out_t = nc.dram_tensor("out", [B, SLAB], f32, kind="ExternalOutput")

    h1p = [nc.dram_tensor(f"h1p{g}", [SLAB, D], FEAT) for g in range(NG)]
    h1pf = [
        nc.dram_tensor(f"h1pf{g}", [SLAB * NCORES, D], FEAT, addr_space="Shared")
        for g in range(NG)
    ]
    q_slab = nc.dram_tensor("q_slab", [SLAB, B], FEAT)
    q_full = nc.dram_tensor("q_full", [SLAB * NCORES, B], FEAT, addr_space="Shared")

    rg = [list(range(NCORES))]

    with tile_mod.TileContext(nc) as tc:
        with (
            tc.tile_pool(name="w", bufs=16) as wp,
            tc.tile_pool(name="gio", bufs=4) as gp,
            tc.tile_pool(name="stab", bufs=2) as sp,
            tc.tile_pool(name="agg", bufs=16) as ap,
            tc.tile_pool(name="small", bufs=3) as mp,
            tc.tile_pool(name="consts", bufs=1) as cp,
            tc.tile_pool(name="pagg", bufs=4, space="PSUM") as pag,
            tc.tile_pool(name="pz", bufs=2, space="PSUM") as pzp,
            tc.tile_pool(name="pq", bufs=2, space="PSUM") as pqp,
        ):
            # --- resident weights ---
            w1sb = []
            for fi in range(NFI1):
                w = wp.tile([P, H], f32r, tag="w", name="wsb")
                nc.sync.dma_start(out=w[:], in_=w1_t[fi * P : (fi + 1) * P, :])
                w1sb.append(w)
            w2sb = []
            for fi in range(NFI2):
                w = wp.tile([P, H], f32r, tag="w", name="wsb")
                nc.sync.dma_start(out=w[:], in_=w2_t[fi * P : (fi + 1) * P, :])
                w2sb.append(w)
            w3sb = []
            for fo in range(NFI2):
                w = wp.tile([P, B], f32r, tag="w3", name="w3sb")
                nc.sync.dma_start(out=w[:], in_=w3i_t[fo * P : (fo + 1) * P, :])
                w3sb.append(w)
            if use_b1:
                b1sb = cp.tile([1, H], f32r)
                nc.sync.dma_start(out=b1sb[:], in_=b1_t[:])
                ones1 = cp.tile([1, P], f32r)
                nc.gpsimd.memset(ones1[:], 1.0)
            if use_b2:
                b2sb = cp.tile([P, NFI2], f32r)
                nc.sync.dma_start(out=b2sb[:], in_=b2_t[:])

            relu = mybir.ActivationFunctionType.Relu

            # ---------------- Layer 1 ----------------
            for t in range(NDT if _phases >= 1 else 0):
                idx_t = mp.tile([P, ECH], i32, tag="idx")
                nc.sync.dma_start(out=idx_t[:], in_=idxA_t[t])
                s_t = sp.tile([P, ECH * DT], FEAT, tag="s")
                nc.sync.dma_start(out=s_t[:], in_=s_tab[t])

                pa = [pag.tile([P, DT], f32, tag="pagg", name="pa") for _ in range(NFI1)]
                for c in range(ECH):
                    g = gp.tile([P, D], FEAT, tag="g")
                    nc.gpsimd.indirect_dma_start(
                        out=g[:],
                        out_offset=None,
                        in_=nodes_t[:],
                        in_offset=bass.IndirectOffsetOnAxis(
                            ap=idx_t[:, c : c + 1], axis=0
                        ),
                    )
                    for fi in range(NFI1):
                        nc.tensor.matmul(
                            out=pa[fi][:],
                            lhsT=g[:, fi * P : (fi + 1) * P],
                            rhs=s_t[:, c * DT : (c + 1) * DT],
                            start=(c == 0),
                            stop=(c == ECH - 1),
                        )
                aggT = []
                for fi in range(NFI1):
                    a = ap.tile([P, DT], f32r, tag="aggT", name="aggTt")
                    nc.vector.tensor_copy(out=a[:], in_=pa[fi][:])
                    aggT.append(a)

                for dh in range(2):
                    for fo in range(NG):  # 4 output blocks of 512
                        pz = pzp.tile([P, D], f32, tag="pz")
                        if use_b1:
                            nc.tensor.matmul(
                                out=pz[:],
                                lhsT=ones1[:1, :],
                                rhs=b1sb[:1, fo * D : (fo + 1) * D],
                                start=True,
                                stop=False,
                            )
                        for fi in range(NFI1):
                            nc.tensor.matmul(
                                out=pz[:],
                                lhsT=aggT[fi][:, dh * P : (dh + 1) * P],
                                rhs=w1sb[fi][:, fo * D : (fo + 1) * D],
                                start=(fi == 0 and not use_b1),
                                stop=(fi == NFI1 - 1),
                            )
                        h_t = mp.tile([P, D], FEAT, tag="hout")
                        nc.scalar.activation(out=h_t[:], in_=pz[:], func=relu)
                        nc.sync.dma_start(
                            out=h1p[fo][t * DT + dh * P : t * DT + (dh + 1) * P, :],
                            in_=h_t[:],
                        )

            for g_i in range(NG if _phases >= 1.5 else 0):
                nc.gpsimd.collective_compute(
                    "AllGather",
                    mybir.AluOpType.bypass,
                    replica_groups=rg,
                    ins=[h1p[g_i][:]],
                    outs=[h1pf[g_i][:]],
                )

            # ---------------- Layer 2 + Q ----------------
            for t in range(NDT if _phases >= 2 else 0):
                idx_t = mp.tile([P, ECH], i32, tag="idx")
                nc.sync.dma_start(out=idx_t[:], in_=idxB_t[t])
                s_t = sp.tile([P, ECH * DT], FEAT, tag="s")
                nc.sync.dma_start(out=s_t[:], in_=s_tab[t])

                aggT = []
                for g_i in range(NG):
                    pa = [pag.tile([P, DT], f32, tag="pagg", name="pa") for _ in range(NFI1)]
                    for c in range(ECH):
                        g = gp.tile([P, D], FEAT, tag="g")
                        nc.gpsimd.indirect_dma_start(
                            out=g[:],
                            out_offset=None,
                            in_=h1pf[g_i][:],
                            in_offset=bass.IndirectOffsetOnAxis(
                                ap=idx_t[:, c : c + 1], axis=0
                            ),
                        )
                        for j in range(NFI1):
                            nc.tensor.matmul(
                                out=pa[j][:],
                                lhsT=g[:, j * P : (j + 1) * P],
                                rhs=s_t[:, c * DT : (c + 1) * DT],
                                start=(c == 0),
                                stop=(c == ECH - 1),
                            )
                    for j in range(NFI1):
                        a = ap.tile([P, DT], f32r, tag="aggT", name="aggTt")
                        nc.vector.tensor_copy(out=a[:], in_=pa[j][:])
                        aggT.append(a)

                pq = [pqp.tile([P, B], f32, tag="pq", name="pq") for _ in range(2)]
                for fo in range(NFO2):
                    pz = pzp.tile([P, DT], f32, tag="pz")
                    for fi in range(NFI2):
                        nc.tensor.matmul(
                            out=pz[:],
                            lhsT=w2sb[fi][:, fo * P : (fo + 1) * P],
                            rhs=aggT[fi][:],
                            start=(fi == 0),
                            stop=(fi == NFI2 - 1),
                        )
                    h2t = mp.tile([P, DT], f32r, tag="h2")
                    if use_b2:
                        nc.scalar.activation(
                            out=h2t[:], in_=pz[:], func=relu,
                            bias=b2sb[:, fo : fo + 1],
                        )
                    else:
                        nc.scalar.activation(out=h2t[:], in_=pz[:], func=relu)
                    for dh in range(2):
                        nc.tensor.matmul(
                            out=pq[dh][:],
                            lhsT=h2t[:, dh * P : (dh + 1) * P],
                            rhs=w3sb[fo][:],
                            start=(fo == 0),
                            stop=(fo == NFO2 - 1),
                        )
                for dh in range(2):
                    qn = mp.tile([P, B], FEAT, tag="qn")
                    nc.vector.tensor_copy(out=qn[:], in_=pq[dh][:])
                    nc.sync.dma_start(
                        out=q_slab[t * DT + dh * P : t * DT + (dh + 1) * P, :],
                        in_=qn[:],
                    )

            if _phases >= 2:
                nc.gpsimd.collective_compute(
                    "AllGather",
                    mybir.AluOpType.bypass,
                    replica_groups=rg,
                    ins=[q_slab[:]],
                    outs=[q_full[:]],
                )

            # ---------------- Layer 3 (= output) ----------------
            for t in range(NDT if _phases >= 3 else 0):
                idx_t = mp.tile([P, ECH], i32, tag="idx")
                nc.sync.dma_start(out=idx_t[:], in_=idxB_t[t])
                s_t = sp.tile([P, ECH * DT], FEAT, tag="s")
                nc.sync.dma_start(out=s_t[:], in_=s_tab[t])

                pa = pag.tile([B, DT], f32, tag="pagg", name="pa3")
                for c in range(ECH):
                    g = gp.tile([P, B], FEAT, tag="g")
                    nc.gpsimd.indirect_dma_start(
                        out=g[:],
                        out_offset=None,
                        in_=q_full[:],
                        in_offset=bass.IndirectOffsetOnAxis(
                            ap=idx_t[:, c : c + 1], axis=0
                        ),
                    )
                    nc.tensor.matmul(
                        out=pa[:],
                        lhsT=g[:],
                        rhs=s_t[:, c * DT : (c + 1) * DT],
                        start=(c == 0),
                        stop=(c == ECH - 1),
                    )
                o_t = mp.tile([B, DT], f32, tag="ot")
                nc.vector.tensor_copy(out=o_t[:], in_=pa[:])
                nc.sync.dma_start(out=out_t[:, t * DT : (t + 1) * DT], in_=o_t[:])

    nc.finalize()
    return nc


_CACHE: dict = {}


def kernel(**inputs: np.ndarray) -> np.ndarray:
    nodes = np.asarray(inputs["nodes"], dtype=np.float32)
    edge_index = np.asarray(inputs["edge_index"])
    img = np.asarray(inputs["img"], dtype=np.float32)
    W1 = np.asarray(inputs["W1"], dtype=np.float32)
    b1 = np.asarray(inputs["b1"], dtype=np.float32)
    W2 = np.asarray(inputs["W2"], dtype=np.float32)
    b2 = np.asarray(inputs["b2"], dtype=np.float32)
    W3 = np.asarray(inputs["W3"], dtype=np.float32)
    b3 = np.asarray(inputs["b3"], dtype=np.float32)

    ECH, idxA, idxB, S = _preprocess(edge_index)
    if FEAT_BF16:
        S = S.astype(__import__("ml_dtypes").bfloat16)
    use_b1 = bool(np.any(b1))
    use_b2 = bool(np.any(b2))

    key = (ECH, use_b1, use_b2)
    if key not in _CACHE:
        _CACHE[key] = _build(ECH, use_b1, use_b2)
    nc = _CACHE[key]

    w3img = _round_fp32r(W3.astype(np.float32) @ img.astype(np.float32).T)  # [H, B]
    outbias = img @ b3  # [B]

    import ml_dtypes
    feat_np = ml_dtypes.bfloat16 if FEAT_BF16 else np.float32
    nodes_r = nodes.astype(feat_np) if FEAT_BF16 else _round_fp32r(nodes)
    w1_r = _round_fp32r(W1)
    w2_r = _round_fp32r(W2)
    b1_r = _round_fp32r(b1.reshape(1, H))
    b2_r = _round_fp32r(np.ascontiguousarray(b2.reshape(NFI2, P).T))

    in_maps = []
    for k in range(NCORES):
        in_maps.append(
            {
                "nodes": nodes_r,
                "W1": w1_r,
                "W2": w2_r,
                "W3img": w3img,
                "b1": b1_r,
                "b2": b2_r,
                "idxA": np.ascontiguousarray(idxA[k]),
                "idxB": np.ascontiguousarray(idxB[k]),
                "S": np.ascontiguousarray(S[k]),
            }
        )

    res = run_bass_kernel_spmd(nc, in_maps, core_ids=list(range(NCORES)))

    full = np.concatenate([res.results[k]["out"] for k in range(NCORES)], axis=1)
    n_ids = np.arange(N_SKIP, N)
    cols = (n_ids // NODES_PER) * SLAB + (n_ids % NODES_PER)
    out = full[:, cols] + outbias[:, None]
    return out.astype(np.float32)


if __name__ == "__main__":
    # quick self-exercise with random data
    rng = np.random.default_rng(0)
    ins = {
        "nodes": rng.standard_normal((N, D)).astype(np.float32),
        "edge_index": rng.integers(0, N, size=(2, E)).astype(np.int64),
        "img": rng.standard_normal((B, D)).astype(np.float32),
        "W1": (rng.standard_normal((D, H)) * 0.02).astype(np.float32),
        "b1": np.zeros(H, np.float32),
        "W2": (rng.standard_normal((H, H)) * 0.02).astype(np.float32),
        "b2": np.zeros(H, np.float32),
        "W3": (rng.standard_normal((H, D)) * 0.02).astype(np.float32),
        "b3": np.zeros(D, np.float32),
    }
    out = kernel(**ins)
    print("out", out.shape, out.dtype, np.abs(out).mean())
